# revision 9
# baseline (speedup 1.0000x reference)
"""Trainium2 Bass kernel for nn_CapsuleLayer (dynamic-routing capsule layer).

Reference computation (per batch row b of B=16384):
    hat[b] = (x[b] @ W).reshape(32, 16)          # capsule predictions
    b0 = 0; 3x routing:
        c = softmax(b0 over caps); s = sum_n c_n hat_n; v = squash(s)
        b0 += hat . v
    out[b] = v                                    # [16]

Distribution: data-parallel over batch across 8 NeuronCores (x sharded on
dim 0, W replicated).  Host pre-transposes each x shard so the matmul's
stationary operand (xT tiles) loads directly from DRAM, and appends the
capsule-mean columns of W so the first routing iteration's mean vector m
comes out of the PE for free.

Device layout per core: batch rows on SBUF partitions, the 32x16 capsule
block in the free dimension.  Routing math is restructured so softmax
normalisation and squash scales fold into per-partition scalars:
    v_i = alpha_i * s_i,  s_i = sum_n c_n hat_n = rc_i * sum_n e_n hat_n
so only 4 full-size elementwise passes + 4 grouped reductions per tile are
needed; everything else is O(32) per row.
"""

import sys

sys.path.insert(0, "/opt/trn_rl_repo")

import numpy as np

NUM_CAPSULE = 32
DIM_CAPSULE = 16
EPS = 1e-7

B_FULL = 16384
D_IN = 1024
COLS = NUM_CAPSULE * DIM_CAPSULE  # 512
N_CORES = 8
B_CORE = B_FULL // N_CORES  # 2048
TILES = B_CORE // 128  # 16 batch tiles of 128 rows
KC = D_IN // 128  # 8 contraction chunks
GRP = 8  # tiles per small-op group
WZ_COLS = COLS + DIM_CAPSULE  # W plus capsule-mean columns


_CUSTOM = {}


def _get_mul_pscan():
    """Register (once) a fused custom DVE op:
    out = inclusive_prefix_sum(in0 * in1) along the free-dim stream.
    Page sums then come from differences of page-boundary prefix values,
    fusing what was a tensor_tensor mul + tensor_reduce (2 passes) into
    one DVE pass + tiny boundary ops.
    """
    if "op" in _CUSTOM:
        return _CUSTOM["op"]
    from concourse import dve_ops
    from concourse.dve_spec import AluOp, Spec, Src0, Src1, lower, scan
    from concourse.dve_uop import DveOpSpec

    name = "MUL_PSCAN"
    for op in dve_ops.OPS:
        if op.name == name:
            _CUSTOM["op"] = op
            return op

    def _ref(in0, in1, c0, c1, c2):
        p = np.asarray(in0, np.float32) * np.asarray(in1, np.float32)
        flat = p.reshape(p.shape[0], -1)
        return np.cumsum(flat, axis=1, dtype=np.float32).reshape(p.shape)

    spec = Spec(body=scan(AluOp.ADD, Src0 * Src1), reference=_ref)
    row = max(dve_ops._SUB_OPCODE_FOR_NAME.values()) + 1
    shas = {}
    for ver in ("v3", "v4"):
        try:
            uops = lower(spec, ver=ver)
        except Exception:
            continue
        shas[ver] = DveOpSpec(name=name, opcode=row, uops=uops, rd1_en=True).sha(ver)
    op = dve_ops.DveOp(name, spec, subdim=False, uops_sha=shas)
    dve_ops.OPS.append(op)
    dve_ops._SUB_OPCODE_FOR_NAME[name] = row
    dve_ops.CUSTOM_DVE_SPECS[name] = spec
    _CUSTOM["op"] = op
    return op


def _build_program():
    import concourse.bass as bass
    import concourse.mybir as mybir
    import concourse.tile as tile
    from concourse import bacc

    f32 = mybir.dt.float32
    f32r = mybir.dt.float32r
    AX = mybir.AxisListType.X
    ADD = mybir.AluOpType.add
    ACT = mybir.ActivationFunctionType

    scan_op = _get_mul_pscan()

    nc = bacc.Bacc("TRN2", target_bir_lowering=False, debug=False)

    xt = nc.dram_tensor("xt", [D_IN, B_CORE], f32r, kind="ExternalInput").ap()
    wz = nc.dram_tensor("wz", [D_IN, WZ_COLS], f32r, kind="ExternalInput").ap()
    out = nc.dram_tensor(
        "out", [128, TILES * DIM_CAPSULE], f32, kind="ExternalOutput"
    ).ap()

    xt_v = xt.rearrange("(k p) b -> p k b", p=128)
    wz_v = wz.rearrange("(k p) c -> p k c", p=128)

    with tile.TileContext(nc) as tc:
        with (
            tc.tile_pool(name="w", bufs=1) as wp,
            tc.tile_pool(name="x", bufs=3) as xp,
            tc.tile_pool(name="hat", bufs=TILES) as hatp,
            tc.tile_pool(name="m", bufs=TILES) as mp,
            tc.tile_pool(name="grp", bufs=2) as gp,
            tc.tile_pool(name="scan", bufs=2) as scanp,
            tc.tile_pool(name="stat", bufs=1) as st,
            tc.tile_pool(name="psum", bufs=3, space="PSUM") as pp,
            tc.tile_pool(name="psum_m", bufs=2, space="PSUM") as pmp,
        ):
            # --- replicated weights (one tile per K chunk for fine deps) ---
            wk = []
            for k in range(KC):
                w_t = wp.tile([128, WZ_COLS], f32r, tag=f"w{k}")
                nc.sync.dma_start(w_t[:], wz_v[:, k, :])
                wk.append(w_t)

            # persistent per-core tensors
            hat = []  # 16 x [128, 512]
            m_t = []  # 16 x [128, 16]
            b1r = st.tile([128, TILES, NUM_CAPSULE], f32)  # raw hat.m dots
            b1s = st.tile([128, TILES, NUM_CAPSULE], f32)  # scaled logits it1
            e2 = st.tile([128, TILES, NUM_CAPSULE], f32)
            e3 = st.tile([128, TILES, NUM_CAPSULE], f32)
            h2r = st.tile([128, TILES, NUM_CAPSULE], f32)
            b2 = st.tile([128, TILES, NUM_CAPSULE], f32)
            s2r = st.tile([128, TILES, DIM_CAPSULE], f32)
            s3r = st.tile([128, TILES, DIM_CAPSULE], f32)
            outb = st.tile([128, TILES, DIM_CAPSULE], f32)

            def matmul_tile(t):
                xs = xp.tile([128, KC, 128], f32r, tag="xs")
                nc.sync.dma_start(xs[:], xt_v[:, :, bass.ts(t, 128)])
                hp = pp.tile([128, COLS], f32, tag="hp")
                mps = pmp.tile([128, DIM_CAPSULE], f32, tag="mp")
                for k in range(KC):
                    nc.tensor.matmul(
                        hp[:],
                        xs[:, k, :],
                        wk[k][:, 0:COLS],
                        start=(k == 0),
                        stop=(k == KC - 1),
                    )
                for k in range(KC):
                    nc.tensor.matmul(
                        mps[:],
                        xs[:, k, :],
                        wk[k][:, COLS:WZ_COLS],
                        start=(k == 0),
                        stop=(k == KC - 1),
                    )
                h_t = hatp.tile([128, COLS], f32, tag="hat")
                nc.scalar.copy(h_t[:], hp[:])
                mt = mp.tile([128, DIM_CAPSULE], f32, tag="m")
                nc.scalar.copy(mt[:], mps[:])
                hat.append(h_t)
                m_t.append(mt)

            def nd(ap):  # [128, 512] -> [128, 32, 16] (caps, dim)
                return ap.rearrange("p (n d) -> p n d", d=DIM_CAPSULE)

            def dn(ap):  # [128, 512] -> [128, 16, 32] (dim-major view)
                return ap.rearrange("p (n d) -> p d n", d=DIM_CAPSULE)

            def bc_caps(ap):  # [128, 16] -> [128, 32, 16]
                return ap.unsqueeze(1).broadcast_to([128, NUM_CAPSULE, DIM_CAPSULE])

            def bc_dim(ap):  # [128, 32] -> [128, 16, 32]
                return ap.unsqueeze(1).broadcast_to([128, DIM_CAPSULE, NUM_CAPSULE])

            def bc_g(ap, n):  # [128, G] -> [128, G, n]
                return ap.unsqueeze(2).broadcast_to([128, GRP, n])

            def scan_caps(t, i, in1_16, scr):
                """scr[:, i] = prefix(hat_t * bc(in1_16)), stream n-major."""
                nc.vector._custom_dve(
                    scan_op,
                    out=nd(scr[:, i, :]),
                    in0=nd(hat[t][:]),
                    in1=bc_caps(in1_16),
                )

            def scan_dims(t, i, in1_32, scr):
                """scr[:, i] = prefix(hat_t * bc(in1_32)), stream d-major."""
                nc.vector._custom_dve(
                    scan_op,
                    out=dn(scr[:, i, :]),
                    in0=dn(hat[t][:]),
                    in1=bc_dim(in1_32),
                )

            def diffs_caps(scr, out_g):
                """out_g[:, :, n] = page sums over d from caps-major scans."""
                ends = scr[:].rearrange("p g (n d) -> p g n d", d=DIM_CAPSULE)[
                    :, :, :, DIM_CAPSULE - 1
                ]  # [128, G, 32]
                nc.vector.tensor_copy(out_g[:, :, 0:1], ends[:, :, 0:1])
                nc.vector.tensor_sub(out_g[:, :, 1:], ends[:, :, 1:], ends[:, :, :-1])

            def diffs_dims(scr, out_g):
                """out_g[:, :, d] = page sums over n from dim-major scans."""
                ends = scr[:].rearrange("p g (n d) -> p g d n", d=DIM_CAPSULE)[
                    :, :, :, NUM_CAPSULE - 1
                ]  # [128, G, 16]
                nc.vector.tensor_copy(out_g[:, :, 0:1], ends[:, :, 0:1])
                nc.vector.tensor_sub(out_g[:, :, 1:], ends[:, :, 1:], ends[:, :, :-1])

            def alpha_chain(S2, tag):
                """alpha = S2 / (1 + S2) / sqrt(S2 + EPS), all [128, G]."""
                pe = gp.tile([128, GRP], f32, tag=f"pe{tag}")
                nc.vector.tensor_scalar_add(pe[:], S2, EPS)
                r = gp.tile([128, GRP], f32, tag=f"r{tag}")
                nc.scalar.activation(r[:], pe[:], ACT.Sqrt)
                u = gp.tile([128, GRP], f32, tag=f"u{tag}")
                nc.vector.tensor_scalar_add(u[:], S2, 1.0)
                nc.vector.tensor_mul(u[:], u[:], r[:])
                rc = gp.tile([128, GRP], f32, tag=f"rc{tag}")
                nc.vector.reciprocal(rc[:], u[:])
                a = gp.tile([128, GRP], f32, tag=f"a{tag}")
                nc.vector.tensor_mul(a[:], S2, rc[:])
                return a

            def sq_norm(src_g, width, tag):
                """S2[:, j] = sum over width of src_g[:, j, :]**2  (group)."""
                sq = gp.tile([128, GRP, width], f32, tag=f"sq{tag}")
                nc.vector.tensor_mul(sq[:], src_g, src_g)
                S2 = gp.tile([128, GRP], f32, tag=f"S2{tag}")
                nc.vector.tensor_reduce(S2[:], sq[:], axis=AX, op=ADD)
                return S2

            def softmax_prep(logits_g, e_out_g, tag):
                """e = exp(logits); rc = 1/sum_n e   (group)."""
                nc.scalar.activation(e_out_g, logits_g, ACT.Exp)
                se = gp.tile([128, GRP], f32, tag=f"se{tag}")
                nc.vector.tensor_reduce(se[:], e_out_g, axis=AX, op=ADD)
                rc = gp.tile([128, GRP], f32, tag=f"srtrue{tag}")
                nc.vector.reciprocal(rc[:], se[:])
                return rc

            for g in range(TILES // GRP):
                t0 = g * GRP
                gs = slice(t0, t0 + GRP)
                tiles_g = range(t0, t0 + GRP)

                for t in tiles_g:
                    matmul_tile(t)
                    # iteration 1: b1_raw = hat . m   (uniform-c mean comes
                    # from the appended W columns)
                    dot_caps(t, m_t[t][:], b1r[:, t, :])

                # m tiles -> group view for |m|^2 (per-tile copies into one
                # grouped tensor are cheap [128,16] DVE ops)
                m_g = gp.tile([128, GRP, DIM_CAPSULE], f32, tag="mg")
                for i, t in enumerate(tiles_g):
                    nc.vector.tensor_copy(m_g[:, i, :], m_t[t][:])

                S2_1 = sq_norm(m_g[:], DIM_CAPSULE, "1")
                a1 = alpha_chain(S2_1[:], "1")
                # scaled logits iter1:  b1 = alpha1 * (hat . m)
                nc.vector.tensor_mul(
                    b1s[:, gs, :], b1r[:, gs, :], bc_g(a1[:], NUM_CAPSULE)
                )

                rc2 = softmax_prep(b1s[:, gs, :], e2[:, gs, :], "2")
                for i, t in enumerate(tiles_g):
                    # s2_raw = sum_n e2_n hat_n   (true s2 = rc2 * s2_raw)
                    dot_dims(t, e2[:, t, :], s2r[:, t, :])

                S2r = sq_norm(s2r[:, gs, :], DIM_CAPSULE, "2")
                # |s2|^2 = rc2^2 * |s2_raw|^2
                nc.vector.tensor_mul(S2r[:], S2r[:], rc2[:])
                nc.vector.tensor_mul(S2r[:], S2r[:], rc2[:])
                a2 = alpha_chain(S2r[:], "2")
                a2p = gp.tile([128, GRP], f32, tag="a2p")  # alpha2 * rc2
                nc.vector.tensor_mul(a2p[:], a2[:], rc2[:])

                for i, t in enumerate(tiles_g):
                    # h2_raw = hat . s2_raw ;  hat . v2 = a2p * h2_raw
                    dot_caps(t, s2r[:, t, :], h2r[:, t, :])

                # b2 = b1 + a2p * h2_raw
                tmp = gp.tile([128, GRP, NUM_CAPSULE], f32, tag="tmp2")
                nc.vector.tensor_mul(tmp[:], h2r[:, gs, :], bc_g(a2p[:], NUM_CAPSULE))
                nc.vector.tensor_add(b2[:, gs, :], b1s[:, gs, :], tmp[:])

                rc3 = softmax_prep(b2[:, gs, :], e3[:, gs, :], "3")
                for i, t in enumerate(tiles_g):
                    dot_dims(t, e3[:, t, :], s3r[:, t, :])

                S3r = sq_norm(s3r[:, gs, :], DIM_CAPSULE, "3")
                nc.vector.tensor_mul(S3r[:], S3r[:], rc3[:])
                nc.vector.tensor_mul(S3r[:], S3r[:], rc3[:])
                a3 = alpha_chain(S3r[:], "3")
                a3p = gp.tile([128, GRP], f32, tag="a3p")
                nc.vector.tensor_mul(a3p[:], a3[:], rc3[:])
                # v3 = a3p * s3_raw  -> output
                nc.vector.tensor_mul(
                    outb[:, gs, :], s3r[:, gs, :], bc_g(a3p[:], DIM_CAPSULE)
                )

            nc.sync.dma_start(out[:], outb[:].rearrange("p t d -> p (t d)"))

    nc.compile()
    return nc


_PROGRAM_CACHE = {}


def _get_program():
    if "nc" not in _PROGRAM_CACHE:
        _PROGRAM_CACHE["nc"] = _build_program()
    return _PROGRAM_CACHE["nc"]


def _host_prep(x, w):
    """Shard + transpose x, extend W with capsule-mean columns."""
    x = np.ascontiguousarray(x, dtype=np.float32)
    w = np.ascontiguousarray(w, dtype=np.float32)
    wavg = w.reshape(D_IN, NUM_CAPSULE, DIM_CAPSULE).mean(axis=1)
    wz = np.ascontiguousarray(np.concatenate([w, wavg], axis=1))
    in_maps = []
    for i in range(N_CORES):
        shard = x[i * B_CORE : (i + 1) * B_CORE]  # [2048, 1024]
        xt = np.ascontiguousarray(shard.T)  # [1024, 2048]
        in_maps.append({"xt": xt, "wz": wz})
    return in_maps


def _gather(results):
    outs = []
    for i in range(N_CORES):
        o = results[i]["out"]  # [128, 256]
        o = o.reshape(128, TILES, DIM_CAPSULE).transpose(1, 0, 2)
        outs.append(o.reshape(B_CORE, DIM_CAPSULE))
    return np.ascontiguousarray(np.concatenate(outs, axis=0))


def run_on_hw(x, w, trace=False):
    """Returns (output [16384, 16], BassKernelResults)."""
    from concourse import bass_utils

    nc = _get_program()
    in_maps = _host_prep(x, w)
    res = bass_utils.run_bass_kernel_spmd(
        nc, in_maps, core_ids=list(range(N_CORES)), trace=trace
    )
    return _gather(res.results), res


def kernel(**inputs):
    out, _ = run_on_hw(inputs["x"], inputs["kernel"])
    return out


# revision 11
# speedup vs baseline: 1.3260x; 1.3260x over previous
"""Trainium2 Bass kernel for nn_CapsuleLayer (dynamic-routing capsule layer).

Reference computation (per batch row b of B=16384):
    hat[b] = (x[b] @ W).reshape(32, 16)          # capsule predictions
    b0 = 0; 3x routing:
        c = softmax(b0 over caps); s = sum_n c_n hat_n; v = squash(s)
        b0 += hat . v
    out[b] = v                                    # [16]

Distribution: data-parallel over batch across 8 NeuronCores (x sharded on
dim 0, W replicated).  Host pre-transposes each x shard so the matmul's
stationary operand (xT tiles) loads directly from DRAM; W's columns are
reordered d-major so the PSUM result is already in dim-major layout.

Device layout per core: batch rows on SBUF partitions, the 32x16 capsule
block in the free dimension, kept in BOTH d-major and n-major copies so
every large vector op streams with unit stride.  All capsule-space
reductions are fused multiply+prefix-scan custom DVE ops; page sums come
from differences of page-boundary prefix values.  Softmax normalisation
and squash scales fold into per-partition scalars, so everything outside
the five full-size scans is O(32) per row.
"""

import sys

sys.path.insert(0, "/opt/trn_rl_repo")

import numpy as np

NUM_CAPSULE = 32
DIM_CAPSULE = 16
EPS = 1e-7

B_FULL = 16384
D_IN = 1024
COLS = NUM_CAPSULE * DIM_CAPSULE  # 512
N_CORES = 8
B_CORE = B_FULL // N_CORES  # 2048
TILES = B_CORE // 128  # 16 batch tiles of 128 rows
KC = D_IN // 128  # 8 contraction chunks
GRP = 8  # tiles per small-op group

_CUSTOM = {}


def _register_scan_ops():
    """Register (once) two custom DVE ops:
    MUL_PSCAN: out = inclusive_prefix_sum(in0 * in1) along the stream.
    PSCAN:     out = inclusive_prefix_sum(in0).
    Page sums then come from differences of page-boundary prefix values,
    fusing what was a tensor_tensor mul + tensor_reduce (2 passes) into
    one DVE pass + tiny boundary ops.
    """
    if "mul" in _CUSTOM:
        return _CUSTOM["mul"], _CUSTOM["plain"]
    from concourse import dve_ops
    from concourse.dve_spec import AluOp, Spec, Src0, Src1, lower, scan
    from concourse.dve_uop import DveOpSpec

    def _make(name, body, ref, rd1):
        for op in dve_ops.OPS:
            if op.name == name:
                return op
        spec = Spec(body=body, reference=ref)
        row = max(dve_ops._SUB_OPCODE_FOR_NAME.values()) + 1
        shas = {}
        for ver in ("v3", "v4"):
            try:
                uops = lower(spec, ver=ver)
            except Exception:
                continue
            shas[ver] = DveOpSpec(name=name, opcode=row, uops=uops, rd1_en=rd1).sha(
                ver
            )
        op = dve_ops.DveOp(name, spec, subdim=False, uops_sha=shas)
        dve_ops.OPS.append(op)
        dve_ops._SUB_OPCODE_FOR_NAME[name] = row
        dve_ops.CUSTOM_DVE_SPECS[name] = spec
        return op

    def _ref_mul(in0, in1, c0, c1, c2):
        a = np.asarray(in0, np.float32).reshape(in0.shape[0], -1)
        b = np.asarray(in1, np.float32).reshape(in1.shape[0], -1)
        p = a * b
        return np.cumsum(p, axis=1, dtype=np.float32).reshape(in0.shape)

    def _ref_plain(in0, in1, c0, c1, c2):
        p = np.asarray(in0, np.float32)
        flat = p.reshape(p.shape[0], -1)
        return np.cumsum(flat, axis=1, dtype=np.float32).reshape(p.shape)

    _CUSTOM["mul"] = _make(
        "MUL_PSCAN", scan(AluOp.ADD, Src0 * Src1), _ref_mul, True
    )
    _CUSTOM["plain"] = _make("PSCAN", scan(AluOp.ADD, Src0), _ref_plain, False)
    return _CUSTOM["mul"], _CUSTOM["plain"]


def _build_program():
    import concourse.bass as bass
    import concourse.mybir as mybir
    import concourse.tile as tile
    from concourse import bacc

    f32 = mybir.dt.float32
    f32r = mybir.dt.float32r
    AX = mybir.AxisListType.X
    ADD = mybir.AluOpType.add
    ACT = mybir.ActivationFunctionType

    mul_scan, plain_scan = _register_scan_ops()

    nc = bacc.Bacc("TRN2", target_bir_lowering=False, debug=False)

    xt = nc.dram_tensor("xt", [D_IN, B_CORE], f32r, kind="ExternalInput").ap()
    # W with columns reordered d-major: wz[:, d*32+n] = W[:, n*16+d]
    wz = nc.dram_tensor("wz", [D_IN, COLS], f32r, kind="ExternalInput").ap()
    out = nc.dram_tensor(
        "out", [128, TILES * DIM_CAPSULE], f32, kind="ExternalOutput"
    ).ap()

    xt_v = xt.rearrange("(k p) b -> p k b", p=128)
    wz_v = wz.rearrange("(k p) c -> p k c", p=128)

    with tile.TileContext(nc) as tc:
        with (
            tc.tile_pool(name="w", bufs=1) as wp,
            tc.tile_pool(name="x", bufs=3) as xp,
            tc.tile_pool(name="hat", bufs=TILES) as hatp,
            tc.tile_pool(name="grp", bufs=2) as gp,
            tc.tile_pool(name="scan", bufs=2) as scanp,
            tc.tile_pool(name="stat", bufs=1) as st,
            tc.tile_pool(name="psum", bufs=4, space="PSUM") as pp,
        ):
            # --- replicated weights (one tile per K chunk for fine deps) ---
            wk = []
            for k in range(KC):
                w_t = wp.tile([128, COLS], f32r, tag=f"w{k}")
                nc.sync.dma_start(w_t[:], wz_v[:, k, :])
                wk.append(w_t)

            # persistent per-core tensors (hat in both layouts)
            hat_dn = []  # 16 x [128, 512]  free = (d, n), n contiguous
            hat_nd = []  # 16 x [128, 512]  free = (n, d), d contiguous
            b1r = st.tile([128, TILES, NUM_CAPSULE], f32)
            b1s = st.tile([128, TILES, NUM_CAPSULE], f32)
            e2 = st.tile([128, TILES, NUM_CAPSULE], f32)
            e3 = st.tile([128, TILES, NUM_CAPSULE], f32)
            h2r = st.tile([128, TILES, NUM_CAPSULE], f32)
            b2 = st.tile([128, TILES, NUM_CAPSULE], f32)
            s2r = st.tile([128, TILES, DIM_CAPSULE], f32)
            s3r = st.tile([128, TILES, DIM_CAPSULE], f32)
            mg = st.tile([128, TILES, DIM_CAPSULE], f32)  # 32*m per tile
            outb = st.tile([128, TILES, DIM_CAPSULE], f32)

            def matmul_tile(t):
                xs = xp.tile([128, KC, 128], f32r, tag="xs")
                nc.sync.dma_start(xs[:], xt_v[:, :, bass.ts(t, 128)])
                hp = pp.tile([128, COLS], f32, tag="hp")
                for k in range(KC):
                    nc.tensor.matmul(
                        hp[:],
                        xs[:, k, :],
                        wk[k][:],
                        start=(k == 0),
                        stop=(k == KC - 1),
                    )
                h_dn = hatp.tile([128, COLS], f32, tag="hdn")
                nc.scalar.copy(h_dn[:], hp[:])
                h_nd = hatp.tile([128, COLS], f32, tag="hnd")
                # permuted copy: h_nd[p, n*16+d] = hp[p, d*32+n]
                nc.scalar.copy(
                    h_nd[:].rearrange("p (n d) -> p d n", d=DIM_CAPSULE), hp[:]
                )
                hat_dn.append(h_dn)
                hat_nd.append(h_nd)

            def nd(ap):  # [128, 512] (n-major buffer) -> [128, 32, 16]
                return ap.rearrange("p (n d) -> p n d", d=DIM_CAPSULE)

            def bc_caps(ap):  # [128, 16] -> [128, 32, 16] (for n-major stream)
                return ap.unsqueeze(1).broadcast_to([128, NUM_CAPSULE, DIM_CAPSULE])

            def bc_dims(ap):  # [128, 32] -> [128, 16, 32] (for d-major stream)
                return ap.unsqueeze(1).broadcast_to([128, DIM_CAPSULE, NUM_CAPSULE])

            def bc_g(ap, n):  # [128, G] -> [128, G, n]
                return ap.unsqueeze(2).broadcast_to([128, GRP, n])

            def scan_caps(t, i, in1_16, scr):
                """prefix(hat ⊙ bc(in1)), n-major stream: page sums over d."""
                nc.vector._custom_dve(
                    mul_scan,
                    out=scr[:, i, :],
                    in0=hat_nd[t][:],
                    in1=bc_caps(in1_16),
                )

            def scan_dims(t, i, in1_32, scr):
                """prefix(hat ⊙ bc(in1)), d-major stream: page sums over n."""
                nc.vector._custom_dve(
                    mul_scan,
                    out=scr[:, i, :],
                    in0=hat_dn[t][:],
                    in1=bc_dims(in1_32),
                )

            def diffs(scr, out_g, width, pages):
                """out_g[:, :, j] = prefix[page j end] - prefix[page j-1 end]."""
                ends = scr[:].rearrange("p g (a b) -> p g a b", b=width)[
                    :, :, :, width - 1
                ]  # [128, G, pages]
                nc.vector.tensor_copy(out_g[:, :, 0:1], ends[:, :, 0:1])
                nc.vector.tensor_sub(out_g[:, :, 1:], ends[:, :, 1:], ends[:, :, :-1])

            def alpha_chain(S2, tag):
                """alpha = S2 / (1 + S2) / sqrt(S2 + EPS), all [128, G]."""
                pe = gp.tile([128, GRP], f32, tag=f"pe{tag}")
                nc.vector.tensor_scalar_add(pe[:], S2, EPS)
                r = gp.tile([128, GRP], f32, tag=f"r{tag}")
                nc.scalar.activation(r[:], pe[:], ACT.Sqrt)
                u = gp.tile([128, GRP], f32, tag=f"u{tag}")
                nc.vector.tensor_scalar_add(u[:], S2, 1.0)
                nc.vector.tensor_mul(u[:], u[:], r[:])
                rc = gp.tile([128, GRP], f32, tag=f"rc{tag}")
                nc.vector.reciprocal(rc[:], u[:])
                a = gp.tile([128, GRP], f32, tag=f"a{tag}")
                nc.vector.tensor_mul(a[:], S2, rc[:])
                return a

            def sq_norm(src_g, width, tag, post_scale=None):
                """S2[:, j] = sum over width of src_g[:, j, :]**2  (group)."""
                sq = gp.tile([128, GRP, width], f32, tag=f"sq{tag}")
                nc.vector.tensor_mul(sq[:], src_g, src_g)
                S2 = gp.tile([128, GRP], f32, tag=f"S2{tag}")
                nc.vector.tensor_reduce(S2[:], sq[:], axis=AX, op=ADD)
                if post_scale is not None:
                    nc.vector.tensor_scalar_mul(S2[:], S2[:], post_scale)
                return S2

            def softmax_prep(logits_g, e_out_g, tag):
                """e = exp(logits); rc = 1/sum_n e   (group)."""
                nc.scalar.activation(e_out_g, logits_g, ACT.Exp)
                se = gp.tile([128, GRP], f32, tag=f"se{tag}")
                nc.vector.tensor_reduce(se[:], e_out_g, axis=AX, op=ADD)
                rc = gp.tile([128, GRP], f32, tag=f"srtrue{tag}")
                nc.vector.reciprocal(rc[:], se[:])
                return rc

            for g in range(TILES // GRP):
                t0 = g * GRP
                gs = slice(t0, t0 + GRP)
                tiles_g = range(t0, t0 + GRP)

                # matmuls + M = sum_n hat_n (plain scan over d-major stream)
                scr = scanp.tile([128, GRP, COLS], f32, tag="scr")
                for i, t in enumerate(tiles_g):
                    matmul_tile(t)
                    nc.vector._custom_dve(
                        plain_scan, out=scr[:, i, :], in0=hat_dn[t][:]
                    )
                diffs(scr, mg[:, gs, :], NUM_CAPSULE, DIM_CAPSULE)

                # iteration 1: b1_raw = hat . M  (true b1 = alpha1/32 * b1_raw)
                scr = scanp.tile([128, GRP, COLS], f32, tag="scr")
                for i, t in enumerate(tiles_g):
                    scan_caps(t, i, mg[:, t, :], scr)
                diffs(scr, b1r[:, gs, :], DIM_CAPSULE, NUM_CAPSULE)

                # |m|^2 = |M|^2 / 1024
                S2_1 = sq_norm(mg[:, gs, :], DIM_CAPSULE, "1", post_scale=1.0 / 1024)
                a1 = alpha_chain(S2_1[:], "1")
                a1b = gp.tile([128, GRP], f32, tag="a1b")
                nc.vector.tensor_scalar_mul(a1b[:], a1[:], 1.0 / NUM_CAPSULE)
                nc.vector.tensor_mul(
                    b1s[:, gs, :], b1r[:, gs, :], bc_g(a1b[:], NUM_CAPSULE)
                )

                rc2 = softmax_prep(b1s[:, gs, :], e2[:, gs, :], "2")
                scr = scanp.tile([128, GRP, COLS], f32, tag="scr")
                for i, t in enumerate(tiles_g):
                    # s2_raw = sum_n e2_n hat_n   (true s2 = rc2 * s2_raw)
                    scan_dims(t, i, e2[:, t, :], scr)
                diffs(scr, s2r[:, gs, :], NUM_CAPSULE, DIM_CAPSULE)

                S2r = sq_norm(s2r[:, gs, :], DIM_CAPSULE, "2")
                nc.vector.tensor_mul(S2r[:], S2r[:], rc2[:])
                nc.vector.tensor_mul(S2r[:], S2r[:], rc2[:])
                a2 = alpha_chain(S2r[:], "2")
                a2p = gp.tile([128, GRP], f32, tag="a2p")  # alpha2 * rc2
                nc.vector.tensor_mul(a2p[:], a2[:], rc2[:])

                scr = scanp.tile([128, GRP, COLS], f32, tag="scr")
                for i, t in enumerate(tiles_g):
                    # h2_raw = hat . s2_raw ;  hat . v2 = a2p * h2_raw
                    scan_caps(t, i, s2r[:, t, :], scr)
                diffs(scr, h2r[:, gs, :], DIM_CAPSULE, NUM_CAPSULE)

                # b2 = b1 + a2p * h2_raw
                tmp = gp.tile([128, GRP, NUM_CAPSULE], f32, tag="tmp2")
                nc.vector.tensor_mul(tmp[:], h2r[:, gs, :], bc_g(a2p[:], NUM_CAPSULE))
                nc.vector.tensor_add(b2[:, gs, :], b1s[:, gs, :], tmp[:])

                rc3 = softmax_prep(b2[:, gs, :], e3[:, gs, :], "3")
                scr = scanp.tile([128, GRP, COLS], f32, tag="scr")
                for i, t in enumerate(tiles_g):
                    scan_dims(t, i, e3[:, t, :], scr)
                diffs(scr, s3r[:, gs, :], NUM_CAPSULE, DIM_CAPSULE)

                S3r = sq_norm(s3r[:, gs, :], DIM_CAPSULE, "3")
                nc.vector.tensor_mul(S3r[:], S3r[:], rc3[:])
                nc.vector.tensor_mul(S3r[:], S3r[:], rc3[:])
                a3 = alpha_chain(S3r[:], "3")
                a3p = gp.tile([128, GRP], f32, tag="a3p")
                nc.vector.tensor_mul(a3p[:], a3[:], rc3[:])
                # v3 = a3p * s3_raw  -> output
                nc.vector.tensor_mul(
                    outb[:, gs, :], s3r[:, gs, :], bc_g(a3p[:], DIM_CAPSULE)
                )

            nc.sync.dma_start(out[:], outb[:].rearrange("p t d -> p (t d)"))

    nc.compile()
    return nc


_PROGRAM_CACHE = {}


def _get_program():
    if "nc" not in _PROGRAM_CACHE:
        _PROGRAM_CACHE["nc"] = _build_program()
    return _PROGRAM_CACHE["nc"]


def _host_prep(x, w):
    """Shard + transpose x; reorder W columns d-major."""
    x = np.ascontiguousarray(x, dtype=np.float32)
    w = np.ascontiguousarray(w, dtype=np.float32)
    wz = np.ascontiguousarray(
        w.reshape(D_IN, NUM_CAPSULE, DIM_CAPSULE).transpose(0, 2, 1).reshape(
            D_IN, COLS
        )
    )
    in_maps = []
    for i in range(N_CORES):
        shard = x[i * B_CORE : (i + 1) * B_CORE]  # [2048, 1024]
        xtr = np.ascontiguousarray(shard.T)  # [1024, 2048]
        in_maps.append({"xt": xtr, "wz": wz})
    return in_maps


def _gather(results):
    outs = []
    for i in range(N_CORES):
        o = results[i]["out"]  # [128, 256]
        o = o.reshape(128, TILES, DIM_CAPSULE).transpose(1, 0, 2)
        outs.append(o.reshape(B_CORE, DIM_CAPSULE))
    return np.ascontiguousarray(np.concatenate(outs, axis=0))


def run_on_hw(x, w, trace=False):
    """Returns (output [16384, 16], BassKernelResults)."""
    from concourse import bass_utils

    nc = _get_program()
    in_maps = _host_prep(x, w)
    res = bass_utils.run_bass_kernel_spmd(
        nc, in_maps, core_ids=list(range(N_CORES)), trace=trace
    )
    return _gather(res.results), res


def kernel(**inputs):
    out, _ = run_on_hw(inputs["x"], inputs["kernel"])
    return out
